# revision 27
# baseline (speedup 1.0000x reference)
"""Trainium2 Bass kernel for nn_CycleNet_EPD (ragged graph edge-phase decoder).

Math (per graph b, La = edge_len[b], Ba = beta_len[b]):
  ef[e,:4]   = [x[src_e], x[dst_e]]
  s[beta,:]  = sum_e |SCB[b,beta,e]| * ef[e,:]
  emb        = relu(s@W1+b1)@W2+b2 ;  A = emb@W3[:64] + b3
  z_b[:,e]   = A[:,b] + |SCB[b,b,e]| * (W3[64:]^T ef[e])
  H[e,:]     = sum_b relu(z_b)
  out[e,:]   = relu((H@W4 + vb)@W5+b5)@W6+b6,  rows e >= La zero
               vb = 64*b4 + (64-Ba)*relu(A_pad)@W4

v4 design (measured-HW-calibrated):
  - z_b via one K=128 fp8 DoubleRow matmul per beta with CONSTANT masked
    weights (slot s=b%32 occupies rows 2s..2s+1; the 4 edge features ride
    the two k-tiles).  esp[2s+k, kt, e] = fp8(|scb|*ef[2k+kt]) is host-
    packed; betas >=32 use a second esp column layer.  K=128 avoids the
    2x-slow 32-row tile path; DR halves the rhs bytes.
  - relu+bias split over two streams chosen per beta (fixed per slice):
      ACT:  r_b = Relu(pz + A_b)    -> fp8 pair slots; deferred (W4hi,W4hi)
            + (W4lo,W4lo) DoubleRow matmuls fold pairs into PSUM (hi/lo
            keeps W4 at ~bf16 accuracy).
      DVE:  acc = max(pz, -A_b) + acc   (one fused scalar_tensor_tensor,
            using relu(z+A) = max(z,-A) + A; the sum of A's is folded into
            vb on the host).  acc joins via one fp32r matmul per chunk.
  - software pipeline: the in-order PE never waits on a fresh relu (W4
    folds deferred by DEPTH pairs; out-stage ops of chunk k emitted one
    per pair-slot during chunk k+1).
  - out stage: r5 = pW4+vb (DVE), W5 (bf16), r6 = relu(+b5) (ACT), W6
    (bf16), out = +b6 (ACT, fp32); host transposes rows on unshard.

Sharding: per-core (graph, e0, e1) slices balanced by (Ba+OUT_W)*ne+GFIX;
one NEFF with a partition-id switch.  Host does gathers/abs/products/
packing/casts plus the tiny O(Ba*128) prolog (s, emb, A, vb); all
O(Ba*La*128) math runs on the device.
"""

import os
import sys

sys.path.insert(0, "/opt/trn_rl_repo")

import ml_dtypes
import numpy as np

import concourse.bacc as bacc
import concourse.mybir as mybir
import concourse.tile as tile
from concourse import bass_utils

B, MAX_N, MAX_E, MAX_BETA = 16, 512, 1024, 64
NODE_F, HID = 2, 128
NCORES = 8
F32 = mybir.dt.float32
F32R = mybir.dt.float32r
BF16 = mybir.dt.bfloat16
F8 = mybir.dt.float8e4
AF = mybir.ActivationFunctionType
ALU = mybir.AluOpType
PM = mybir.MatmulPerfMode
NPBF16 = ml_dtypes.bfloat16
NP8 = ml_dtypes.float8_e4m3

ECHUNK = 512
OUT_W = 8      # planner: out-stage cost per edge column (beta-col units)
GFIX = 2000    # planner: per-slice fixed cost
ALPHA = 0.52   # fraction of betas on the ACT stream (rest on DVE)
DEPTH = 2      # pairs of deferred W4 folding


def _q8(a):
    return np.clip(a, -240, 240).astype(NP8)


def _assign(ba):
    """Deterministic per-slice engine walk: True = ACT stream."""
    credit, out = 0.0, []
    for _ in range(ba):
        credit += ALPHA
        if credit >= 1.0:
            credit -= 1.0
            out.append(True)
        else:
            out.append(False)
    return out


def _plan(edge_len, beta_len):
    La = [max(1, min(MAX_E, int(v))) for v in edge_len]
    Ba = [max(1, min(MAX_BETA, int(v))) for v in beta_len]

    def el(g, ne):
        nch = -(-ne // ECHUNK)
        return (Ba[g] + OUT_W) * ne + 350 * Ba[g] * nch + GFIX

    total = sum(el(g, La[g]) for g in range(B))
    target = total / NCORES
    pieces = []
    for g in range(B):
        k = max(1, min(round(el(g, La[g]) / target + 0.25), -(-La[g] // 64)))
        base, rem = divmod(La[g], k)
        e0 = 0
        for j in range(k):
            ne = base + (1 if j < rem else 0)
            pieces.append((g, e0, e0 + ne))
            e0 += ne
    pieces.sort(key=lambda p: -el(p[0], p[2] - p[1]))
    cores = [[] for _ in range(NCORES)]
    loads = [0.0] * NCORES
    for p in pieces:
        c = min(range(NCORES), key=lambda i: loads[i])
        cores[c].append(p)
        loads[c] += el(p[0], p[2] - p[1])
    for _ in range(64):
        cM = max(range(NCORES), key=lambda i: loads[i])
        cm = min(range(NCORES), key=lambda i: loads[i])
        surplus = loads[cM] - loads[cm]
        best = None
        for idx, (g, e0, e1) in enumerate(cores[cM]):
            ne_mv = int((surplus / 2 - GFIX) / (Ba[g] + OUT_W))
            ne_mv = min(ne_mv, e1 - e0 - 64)
            if ne_mv >= 64 and (best is None or ne_mv > best[1]):
                best = (idx, ne_mv)
        if best is None:
            break
        idx, ne_mv = best
        g, e0, e1 = cores[cM][idx]
        cores[cM][idx] = (g, e0, e1 - ne_mv)
        cores[cm].append((g, e1 - ne_mv, e1))
        loads[cM] -= (Ba[g] + OUT_W) * ne_mv
        loads[cm] += el(g, ne_mv)
    return La, Ba, cores


def kernel(x, SCB, edge_index, edge_len, beta_len,
           W1, b1, W2, b2, W3, b3, W4, b4, W5, b5, W6, b6):
    x = np.asarray(x, np.float32)
    SCB = np.asarray(SCB, np.float32)
    edge_index = np.asarray(edge_index, np.int32)
    W1, b1 = np.asarray(W1, np.float32), np.asarray(b1, np.float32)
    W2, b2 = np.asarray(W2, np.float32), np.asarray(b2, np.float32)
    W3, b3 = np.asarray(W3, np.float32), np.asarray(b3, np.float32)
    W4, b4 = np.asarray(W4, np.float32), np.asarray(b4, np.float32)
    W5, b5 = np.asarray(W5, np.float32), np.asarray(b5, np.float32)
    W6, b6 = np.asarray(W6, np.float32), np.asarray(b6, np.float32)
    La, Ba, cores = _plan(np.asarray(edge_len), np.asarray(beta_len))
    W3a, W3b = W3[:64], W3[64:]
    W3b8 = _q8(W3b).astype(np.float32)

    # ---- host prolog per graph ----
    ef_all, A_all, vb_all = [], [], []
    K0 = np.maximum((np.maximum(b1, 0) @ W2 + b2) @ W3a + b3, 0.0)
    for g in range(B):
        la, ba = La[g], Ba[g]
        src, dst = edge_index[g, 0, :la], edge_index[g, 1, :la]
        ef = np.concatenate([x[g][src], x[g][dst]], axis=1)
        ef_all.append(ef)
        asc = np.abs(SCB[g][:ba, :la])
        s = asc @ ef
        emb = np.maximum(s @ W1 + b1, 0.0) @ W2 + b2
        A_all.append(emb @ W3a + b3)                       # [ba,128]
        vb_all.append((64.0 - ba) * (K0 @ W4) + 64.0 * b4)

    # ---- per-core layouts ----
    sched = []
    cmax = omax = amax = nslmax = 1
    for c in range(NCORES):
        eoff = ooff = aoff = 0
        items = []
        for (g, e0, e1) in cores[c]:
            ne = e1 - e0
            nep = -(-ne // 8) * 8
            ba = Ba[g]
            L = -(-ba // 32)
            items.append(dict(g=g, e0=e0, e1=e1, ne=ne, nep=nep, L=L,
                              eoff=eoff, ooff=ooff, aoff=aoff))
            eoff += L * nep
            ooff += nep
            aoff += ba
        sched.append(items)
        cmax = max(cmax, -(-eoff // 8) * 8)
        omax = max(omax, ooff)
        amax = max(amax, aoff)
        nslmax = max(nslmax, len(items))

    # constant masked z-weights: slot s rows 2s..2s+1; ktile pairs carry
    # the 4 edge features
    w3bm = np.zeros((64, 32, 2, 128), NP8)
    for s in range(32):
        w3bm[2 * s + 0, s, 0] = W3b8[0]
        w3bm[2 * s + 0, s, 1] = W3b8[1]
        w3bm[2 * s + 1, s, 0] = W3b8[2]
        w3bm[2 * s + 1, s, 1] = W3b8[3]
    w4hi = _q8(W4)
    w4lo = _q8(W4 - w4hi.astype(np.float32))
    w4dr = np.zeros((128, 2, 2, 128), NP8)
    w4dr[:, 0, 0] = w4hi
    w4dr[:, 0, 1] = w4hi
    w4dr[:, 1, 0] = w4lo
    w4dr[:, 1, 1] = w4lo
    wcst = np.zeros((128, 384), np.float32)
    wcst[:, 0:128] = W5
    wcst[:, 128:256] = W6
    wcst[:, 256:384] = W4

    in_maps = []
    for c in range(NCORES):
        esp = np.zeros((64, 2, cmax), NP8)
        acst = np.zeros((128, amax), np.float32)
        fcst = np.zeros((128, 2 + nslmax), np.float32)
        fcst[:, 0] = b5
        fcst[:, 1] = b6
        for si, it in enumerate(sched[c]):
            g, e0, e1, ne = it["g"], it["e0"], it["e1"], it["ne"]
            ba = Ba[g]
            A = A_all[g]
            asn = _assign(ba)
            # vb'' = vb + (sum of DVE-stream A_b) @ W4
            Adve = sum(A[b] for b in range(ba) if not asn[b])
            if not isinstance(Adve, np.ndarray):
                Adve = np.zeros(128, np.float32)
            fcst[:, 2 + si] = vb_all[g] + Adve @ W4
            for b in range(ba):
                acst[:, it["aoff"] + b] = A[b] if asn[b] else -A[b]
            ef = ef_all[g][e0:e1]
            asc = np.abs(SCB[g][:ba, e0:e1])
            for b in range(ba):
                s_, l = b % 32, b // 32
                col = it["eoff"] + l * it["nep"]
                prod = _q8(asc[b][None, :] * ef.T)      # [4,ne]
                esp[2 * s_ + 0, 0, col:col + ne] = prod[0]
                esp[2 * s_ + 0, 1, col:col + ne] = prod[1]
                esp[2 * s_ + 1, 0, col:col + ne] = prod[2]
                esp[2 * s_ + 1, 1, col:col + ne] = prod[3]
        in_maps.append({
            "esp": esp, "w3bm": w3bm, "w4dr": w4dr,
            "wcst": wcst.astype(np.float32), "acst": acst, "fcst": fcst,
        })

    # ---- build program ----
    one_core = os.environ.get("KERNEL_ONE_CORE")
    ndev = 1 if one_core is not None else NCORES
    nc = bacc.Bacc("TRN2", target_bir_lowering=False, debug=False,
                   num_devices=ndev)
    d_in = {}
    for name, arr in in_maps[0].items():
        dt = {np.dtype(NP8): F8, np.dtype(NPBF16): BF16,
              np.dtype(np.float32): F32}[arr.dtype]
        d_in[name] = nc.dram_tensor(name, list(arr.shape), dt,
                                    kind="ExternalInput")
    d_out = nc.dram_tensor("out", [HID, omax], F32, kind="ExternalOutput")

    with tile.TileContext(nc) as tc:
        pid = nc.partition_id()
        with (
            tc.tile_pool(name="const", bufs=1) as cpool,
            tc.tile_pool(name="sb", bufs=1) as sbp,
            tc.tile_pool(name="psZ", bufs=1, space="PSUM") as psZ,
            tc.tile_pool(name="psW", bufs=1, space="PSUM") as psW,
        ):
            esp_t = cpool.tile([64, 2, cmax], F8, tag="esp")
            w3bm_t = cpool.tile([64, 32, 2, 128], F8, tag="w3bm")
            w4dr_t = cpool.tile([128, 2, 2, 128], F8, tag="w4dr")
            wcst_t = cpool.tile([128, 384], F32, tag="wcst")
            wcb_t = cpool.tile([128, 384], BF16, tag="wcb")
            acst_t = cpool.tile([128, amax], F32, tag="acst")
            fcst_t = cpool.tile([128, 2 + nslmax], F32, tag="fcst")
            zcol_t = cpool.tile([128, ECHUNK], F32, tag="zcol")
            qe0 = [nc.sync, nc.scalar, nc.gpsimd]
            for i in range(8):
                qe0[i % 3].dma_start(
                    w3bm_t[:, 4 * i:4 * i + 4, :, :],
                    d_in["w3bm"].ap()[:, 4 * i:4 * i + 4, :, :])
            nc.sync.dma_start(w4dr_t[:], d_in["w4dr"].ap())
            nc.sync.dma_start(wcst_t[:], d_in["wcst"].ap())
            nc.sync.dma_start(acst_t[:], d_in["acst"].ap())
            nc.sync.dma_start(fcst_t[:], d_in["fcst"].ap())
            nc.gpsimd.memset(zcol_t[:], 0.0)
            nc.vector.tensor_copy(wcb_t[:], wcst_t[:])  # bf16 W5|W6|W4
            b5c = fcst_t[:, 0:1]
            b6c = fcst_t[:, 1:2]
            w5b = wcb_t[:, 0:128]
            w6b = wcb_t[:, 128:256]
            w4b = wcb_t[:, 256:384]

            def build_core(c):
                qeng = [nc.sync, nc.scalar, nc.gpsimd]
                qi = 0
                for si, it in enumerate(sched[c]):
                    e_lo = it["eoff"]
                    ln = it["L"] * it["nep"]
                    step = max(512, -(-ln // 6) // 8 * 8)
                    for p0 in range(0, ln, step):
                        pn = min(step, ln - p0)
                        qeng[qi % 3].dma_start(
                            esp_t[:, :, e_lo + p0:e_lo + p0 + pn],
                            d_in["esp"].ap()[:, :,
                                             e_lo + p0:e_lo + p0 + pn])
                        qi += 1

                chunks = []
                for si, it in enumerate(sched[c]):
                    ba = Ba[it["g"]]
                    lim = os.environ.get("KERNEL_LIMIT_BA")
                    if lim is not None:
                        ba = min(ba, int(lim))
                    for c0 in range(0, it["ne"], ECHUNK):
                        n = min(ECHUNK, it["ne"] - c0)
                        chunks.append((it, si, c0, n, ba))

                pending = []

                def emit_pending():
                    if pending:
                        pending.pop(0)()

                for it, si, c0, n, ba in chunks:
                    vb_col = fcst_t[:, 2 + si:3 + si]
                    asn = _assign(ba)
                    nact = sum(asn)
                    napair = (nact + 1) // 2
                    while len(pending) > max(0, ba - 1):
                        emit_pending()
                    pW4 = psW.tile([128, ECHUNK], F32, tag="pW4", bufs=2)
                    started = [False]

                    inflight = []   # completed ACT rp pairs awaiting W4

                    def emit_w4(last):
                        rp = inflight.pop(0)
                        nc.tensor.matmul(
                            pW4[:, :n], w4dr_t[:, 0, :, :], rp[:, :, :n],
                            start=not started[0], stop=False,
                            perf_mode=PM.DoubleRow)
                        started[0] = True
                        nc.tensor.matmul(
                            pW4[:, :n], w4dr_t[:, 1, :, :], rp[:, :, :n],
                            start=False, stop=last,
                            perf_mode=PM.DoubleRow)

                    acc = None
                    act_i = 0
                    rp_cur = None
                    ndve = ba - nact
                    w4_emitted = 0
                    for b in range(ba):
                        s_, l = b % 32, b // 32
                        ec = it["eoff"] + l * it["nep"] + c0
                        pz = psZ.tile([128, ECHUNK], F32, tag="pz", bufs=5)
                        nc.tensor.matmul(
                            pz[:, :n], w3bm_t[:, s_, :, :],
                            esp_t[:, :, ec:ec + n],
                            start=True, stop=True, perf_mode=PM.DoubleRow)
                        a_col = acst_t[:, it["aoff"] + b:
                                       it["aoff"] + b + 1]
                        if asn[b]:
                            kk = act_i % 2
                            if kk == 0:
                                rp_cur = sbp.tile([128, 2, ECHUNK], F8,
                                                  tag="rp", bufs=6,
                                                  name="rp")
                            nc.scalar.activation(
                                rp_cur[:, kk, :n], pz[:, :n],
                                AF.Relu, bias=a_col, scale=1.0)
                            act_i += 1
                            if kk == 1:
                                inflight.append(rp_cur)
                            elif act_i == nact:   # odd straggler
                                nc.gpsimd.memset(rp_cur[:, 1, :n], 0.0)
                                inflight.append(rp_cur)
                            if len(inflight) > DEPTH:
                                last = (w4_emitted == napair - 1
                                        and ndve == 0)
                                emit_w4(last)
                                w4_emitted += 1
                        else:
                            first = acc is None
                            if first:
                                acc = sbp.tile([128, ECHUNK], F32,
                                               tag="acc", bufs=2,
                                               name="acc")
                            nc.vector.scalar_tensor_tensor(
                                acc[:, :n], pz[:, :n], a_col,
                                zcol_t[:, :n] if first else acc[:, :n],
                                ALU.max, ALU.add)
                        emit_pending()
                    while inflight:
                        last = (w4_emitted == napair - 1 and ndve == 0)
                        emit_w4(last)
                        w4_emitted += 1
                    if ndve > 0:
                        # join the DVE accumulator: pW4 += bf16(acc) @ W4
                        accb = sbp.tile([128, ECHUNK], BF16, tag="accb",
                                        bufs=2, name="accb")
                        nc.scalar.activation(accb[:, :n], acc[:, :n],
                                             AF.Identity, bias=0.0,
                                             scale=1.0)
                        nc.tensor.matmul(
                            pW4[:, :n], w4b, accb[:, :n],
                            start=not started[0], stop=True)
                        started[0] = True

                    # ---- deferred out stage ----
                    def make_out(it=it, c0=c0, n=n, pW4=pW4, vb_col=vb_col):
                        st = {}

                        def s1():
                            st["r5"] = sbp.tile([128, ECHUNK], BF16,
                                                tag="r5", bufs=2, name="r5")
                            nc.vector.tensor_scalar(
                                st["r5"][:, :n], pW4[:, :n], vb_col, None,
                                ALU.add)

                        def s2():
                            st["p2"] = psZ.tile([128, ECHUNK], F32,
                                                tag="pz", bufs=5, name="p2")
                            nc.tensor.matmul(
                                st["p2"][:, :n], w5b, st["r5"][:, :n],
                                start=True, stop=True)

                        def s3():
                            st["r6"] = sbp.tile([128, ECHUNK], BF16,
                                                tag="r6", bufs=2, name="r6")
                            nc.scalar.activation(
                                st["r6"][:, :n], st["p2"][:, :n],
                                AF.Relu, bias=b5c, scale=1.0)

                        def s4():
                            st["p3"] = psZ.tile([128, ECHUNK], F32,
                                                tag="pz", bufs=5, name="p3")
                            nc.tensor.matmul(
                                st["p3"][:, :n], w6b, st["r6"][:, :n],
                                start=True, stop=True)

                        def s5():
                            o_sb = sbp.tile([128, ECHUNK], F32, tag="o",
                                            bufs=2, name="o_sb")
                            nc.scalar.activation(
                                o_sb[:, :n], st["p3"][:, :n],
                                AF.Identity, bias=b6c, scale=1.0)
                            oc = it["ooff"] + c0
                            nc.sync.dma_start(d_out.ap()[:, oc:oc + n],
                                              o_sb[:, :n])

                        return [s1, s2, s3, s4, s5]

                    pending.extend(make_out())
                while pending:
                    pending.pop(0)()

            if one_core is not None:
                build_core(int(one_core))
            else:
                for case in tc.Switch(pid, NCORES):
                    build_core(case)

    global LAST_NC, LAST_INMAPS, LAST_SCHED
    LAST_NC, LAST_INMAPS, LAST_SCHED = nc, in_maps, sched
    if os.environ.get("KERNEL_BUILD_ONLY"):
        return np.zeros((B * MAX_E, HID), np.float32)
    nc.compile()
    trace = bool(os.environ.get("KERNEL_TRACE"))
    run_maps = [in_maps[int(one_core)]] if one_core is not None else in_maps
    res = bass_utils.run_bass_kernel_spmd(
        nc, run_maps, core_ids=list(range(len(run_maps))),
        trace=trace,
        trace_cores=list(range(len(run_maps))) if trace else None,
    )
    global LAST_EXEC_NS, LAST_RESULTS
    LAST_RESULTS = res
    LAST_EXEC_NS = res.exec_time_ns

    out = np.zeros((B * MAX_E, HID), np.float32)
    core_list = [int(one_core)] if one_core is not None else range(NCORES)
    for ci, c in enumerate(core_list):
        oc = res.results[ci]["out"]
        for it in sched[c]:
            g, e0, e1 = it["g"], it["e0"], it["e1"]
            out[g * MAX_E + e0:g * MAX_E + e1] = \
                oc[:, it["ooff"]:it["ooff"] + (e1 - e0)].T
    return out


# revision 28
# speedup vs baseline: 1.0442x; 1.0442x over previous
"""Trainium2 Bass kernel for nn_CycleNet_EPD (ragged graph edge-phase decoder).

Math (per graph b, La = edge_len[b], Ba = beta_len[b]):
  ef[e,:4]   = [x[src_e], x[dst_e]]
  s[beta,:]  = sum_e |SCB[b,beta,e]| * ef[e,:]
  emb        = relu(s@W1+b1)@W2+b2 ;  A = emb@W3[:64] + b3
  z_b[:,e]   = A[:,b] + |SCB[b,b,e]| * (W3[64:]^T ef[e])
  H[e,:]     = sum_b relu(z_b)
  out[e,:]   = relu((H@W4 + vb)@W5+b5)@W6+b6,  rows e >= La zero
               vb = 64*b4 + (64-Ba)*relu(A_pad)@W4

v4 design (measured-HW-calibrated):
  - z_b via one K=128 fp8 DoubleRow matmul per beta with CONSTANT masked
    weights (slot s=b%32 occupies rows 2s..2s+1; the 4 edge features ride
    the two k-tiles).  esp[2s+k, kt, e] = fp8(|scb|*ef[2k+kt]) is host-
    packed; betas >=32 use a second esp column layer.  K=128 avoids the
    2x-slow 32-row tile path; DR halves the rhs bytes.
  - relu+bias split over two streams chosen per beta (fixed per slice):
      ACT:  r_b = Relu(pz + A_b)    -> fp8 pair slots; deferred (W4hi,W4hi)
            + (W4lo,W4lo) DoubleRow matmuls fold pairs into PSUM (hi/lo
            keeps W4 at ~bf16 accuracy).
      DVE:  acc = max(pz, -A_b) + acc   (one fused scalar_tensor_tensor,
            using relu(z+A) = max(z,-A) + A; the sum of A's is folded into
            vb on the host).  acc joins via one fp32r matmul per chunk.
  - software pipeline: the in-order PE never waits on a fresh relu (W4
    folds deferred by DEPTH pairs; out-stage ops of chunk k emitted one
    per pair-slot during chunk k+1).
  - out stage: r5 = pW4+vb (DVE), W5 (bf16), r6 = relu(+b5) (ACT), W6
    (bf16), out = +b6 (ACT, fp32); host transposes rows on unshard.

Sharding: per-core (graph, e0, e1) slices balanced by (Ba+OUT_W)*ne+GFIX;
one NEFF with a partition-id switch.  Host does gathers/abs/products/
packing/casts plus the tiny O(Ba*128) prolog (s, emb, A, vb); all
O(Ba*La*128) math runs on the device.
"""

import os
import sys

sys.path.insert(0, "/opt/trn_rl_repo")

import ml_dtypes
import numpy as np

import concourse.bacc as bacc
import concourse.mybir as mybir
import concourse.tile as tile
from concourse import bass_utils

B, MAX_N, MAX_E, MAX_BETA = 16, 512, 1024, 64
NODE_F, HID = 2, 128
NCORES = 8
F32 = mybir.dt.float32
F32R = mybir.dt.float32r
BF16 = mybir.dt.bfloat16
F8 = mybir.dt.float8e4
AF = mybir.ActivationFunctionType
ALU = mybir.AluOpType
PM = mybir.MatmulPerfMode
NPBF16 = ml_dtypes.bfloat16
NP8 = ml_dtypes.float8_e4m3

ECHUNK = 512
OUT_W = 8      # planner: out-stage cost per edge column (beta-col units)
GFIX = 2000    # planner: per-slice fixed cost
ALPHA = 0.46   # fraction of betas on the ACT stream (rest on DVE)
DEPTH = 2      # pairs of deferred W4 folding


def _q8(a):
    return np.clip(a, -240, 240).astype(NP8)


def _assign(ba):
    """Deterministic per-slice engine walk: True = ACT stream."""
    credit, out = 0.0, []
    for _ in range(ba):
        credit += ALPHA
        if credit >= 1.0:
            credit -= 1.0
            out.append(True)
        else:
            out.append(False)
    return out


def _plan(edge_len, beta_len):
    La = [max(1, min(MAX_E, int(v))) for v in edge_len]
    Ba = [max(1, min(MAX_BETA, int(v))) for v in beta_len]

    def el(g, ne):
        return (Ba[g] + OUT_W) * ne + GFIX

    total = sum(el(g, La[g]) for g in range(B))
    target = total / NCORES
    pieces = []
    for g in range(B):
        k = max(1, min(round(el(g, La[g]) / target + 0.25), -(-La[g] // 64)))
        base, rem = divmod(La[g], k)
        e0 = 0
        for j in range(k):
            ne = base + (1 if j < rem else 0)
            pieces.append((g, e0, e0 + ne))
            e0 += ne
    pieces.sort(key=lambda p: -el(p[0], p[2] - p[1]))
    cores = [[] for _ in range(NCORES)]
    loads = [0.0] * NCORES
    for p in pieces:
        c = min(range(NCORES), key=lambda i: loads[i])
        cores[c].append(p)
        loads[c] += el(p[0], p[2] - p[1])
    for _ in range(64):
        cM = max(range(NCORES), key=lambda i: loads[i])
        cm = min(range(NCORES), key=lambda i: loads[i])
        surplus = loads[cM] - loads[cm]
        best = None
        for idx, (g, e0, e1) in enumerate(cores[cM]):
            ne_mv = int((surplus / 2 - GFIX) / (Ba[g] + OUT_W))
            ne_mv = min(ne_mv, e1 - e0 - 64)
            if ne_mv >= 64 and (best is None or ne_mv > best[1]):
                best = (idx, ne_mv)
        if best is None:
            break
        idx, ne_mv = best
        g, e0, e1 = cores[cM][idx]
        cores[cM][idx] = (g, e0, e1 - ne_mv)
        cores[cm].append((g, e1 - ne_mv, e1))
        loads[cM] -= (Ba[g] + OUT_W) * ne_mv
        loads[cm] += el(g, ne_mv)
    return La, Ba, cores


def kernel(x, SCB, edge_index, edge_len, beta_len,
           W1, b1, W2, b2, W3, b3, W4, b4, W5, b5, W6, b6):
    x = np.asarray(x, np.float32)
    SCB = np.asarray(SCB, np.float32)
    edge_index = np.asarray(edge_index, np.int32)
    W1, b1 = np.asarray(W1, np.float32), np.asarray(b1, np.float32)
    W2, b2 = np.asarray(W2, np.float32), np.asarray(b2, np.float32)
    W3, b3 = np.asarray(W3, np.float32), np.asarray(b3, np.float32)
    W4, b4 = np.asarray(W4, np.float32), np.asarray(b4, np.float32)
    W5, b5 = np.asarray(W5, np.float32), np.asarray(b5, np.float32)
    W6, b6 = np.asarray(W6, np.float32), np.asarray(b6, np.float32)
    La, Ba, cores = _plan(np.asarray(edge_len), np.asarray(beta_len))
    W3a, W3b = W3[:64], W3[64:]
    W3b8 = _q8(W3b).astype(np.float32)

    # ---- host prolog per graph ----
    ef_all, A_all, vb_all = [], [], []
    K0 = np.maximum((np.maximum(b1, 0) @ W2 + b2) @ W3a + b3, 0.0)
    for g in range(B):
        la, ba = La[g], Ba[g]
        src, dst = edge_index[g, 0, :la], edge_index[g, 1, :la]
        ef = np.concatenate([x[g][src], x[g][dst]], axis=1)
        ef_all.append(ef)
        asc = np.abs(SCB[g][:ba, :la])
        s = asc @ ef
        emb = np.maximum(s @ W1 + b1, 0.0) @ W2 + b2
        A_all.append(emb @ W3a + b3)                       # [ba,128]
        vb_all.append((64.0 - ba) * (K0 @ W4) + 64.0 * b4)

    # ---- per-core layouts ----
    sched = []
    cmax = omax = amax = nslmax = 1
    for c in range(NCORES):
        eoff = ooff = aoff = 0
        items = []
        for (g, e0, e1) in cores[c]:
            ne = e1 - e0
            nep = -(-ne // 8) * 8
            ba = Ba[g]
            L = -(-ba // 32)
            items.append(dict(g=g, e0=e0, e1=e1, ne=ne, nep=nep, L=L,
                              eoff=eoff, ooff=ooff, aoff=aoff))
            eoff += L * nep
            ooff += nep
            aoff += ba
        sched.append(items)
        cmax = max(cmax, -(-eoff // 8) * 8)
        omax = max(omax, ooff)
        amax = max(amax, aoff)
        nslmax = max(nslmax, len(items))

    # constant masked z-weights: slot s rows 2s..2s+1; ktile pairs carry
    # the 4 edge features
    w3bm = np.zeros((64, 32, 2, 128), NP8)
    for s in range(32):
        w3bm[2 * s + 0, s, 0] = W3b8[0]
        w3bm[2 * s + 0, s, 1] = W3b8[1]
        w3bm[2 * s + 1, s, 0] = W3b8[2]
        w3bm[2 * s + 1, s, 1] = W3b8[3]
    w4hi = _q8(W4)
    w4lo = _q8(W4 - w4hi.astype(np.float32))
    w4dr = np.zeros((128, 2, 2, 128), NP8)
    w4dr[:, 0, 0] = w4hi
    w4dr[:, 0, 1] = w4hi
    w4dr[:, 1, 0] = w4lo
    w4dr[:, 1, 1] = w4lo
    wcst = np.zeros((128, 384), np.float32)
    wcst[:, 0:128] = W5
    wcst[:, 128:256] = W6
    wcst[:, 256:384] = W4

    in_maps = []
    for c in range(NCORES):
        esp = np.zeros((64, 2, cmax), NP8)
        acst = np.zeros((128, amax), np.float32)
        fcst = np.zeros((128, 2 + nslmax), np.float32)
        fcst[:, 0] = b5
        fcst[:, 1] = b6
        for si, it in enumerate(sched[c]):
            g, e0, e1, ne = it["g"], it["e0"], it["e1"], it["ne"]
            ba = Ba[g]
            A = A_all[g]
            asn = _assign(ba)
            # vb'' = vb + (sum of DVE-stream A_b) @ W4
            Adve = sum(A[b] for b in range(ba) if not asn[b])
            if not isinstance(Adve, np.ndarray):
                Adve = np.zeros(128, np.float32)
            fcst[:, 2 + si] = vb_all[g] + Adve @ W4
            for b in range(ba):
                acst[:, it["aoff"] + b] = A[b] if asn[b] else -A[b]
            ef = ef_all[g][e0:e1]
            asc = np.abs(SCB[g][:ba, e0:e1])
            for b in range(ba):
                s_, l = b % 32, b // 32
                col = it["eoff"] + l * it["nep"]
                prod = _q8(asc[b][None, :] * ef.T)      # [4,ne]
                esp[2 * s_ + 0, 0, col:col + ne] = prod[0]
                esp[2 * s_ + 0, 1, col:col + ne] = prod[1]
                esp[2 * s_ + 1, 0, col:col + ne] = prod[2]
                esp[2 * s_ + 1, 1, col:col + ne] = prod[3]
        in_maps.append({
            "esp": esp, "w3bm": w3bm, "w4dr": w4dr,
            "wcst": wcst.astype(np.float32), "acst": acst, "fcst": fcst,
        })

    # ---- build program ----
    one_core = os.environ.get("KERNEL_ONE_CORE")
    ndev = 1 if one_core is not None else NCORES
    nc = bacc.Bacc("TRN2", target_bir_lowering=False, debug=False,
                   num_devices=ndev)
    d_in = {}
    for name, arr in in_maps[0].items():
        dt = {np.dtype(NP8): F8, np.dtype(NPBF16): BF16,
              np.dtype(np.float32): F32}[arr.dtype]
        d_in[name] = nc.dram_tensor(name, list(arr.shape), dt,
                                    kind="ExternalInput")
    d_out = nc.dram_tensor("out", [HID, omax], F32, kind="ExternalOutput")

    with tile.TileContext(nc) as tc:
        pid = nc.partition_id()
        with (
            tc.tile_pool(name="const", bufs=1) as cpool,
            tc.tile_pool(name="sb", bufs=1) as sbp,
            tc.tile_pool(name="psZ", bufs=1, space="PSUM") as psZ,
            tc.tile_pool(name="psW", bufs=1, space="PSUM") as psW,
        ):
            esp_t = cpool.tile([64, 2, cmax], F8, tag="esp")
            w3bm_t = cpool.tile([64, 32, 2, 128], F8, tag="w3bm")
            w4dr_t = cpool.tile([128, 2, 2, 128], F8, tag="w4dr")
            wcst_t = cpool.tile([128, 384], F32, tag="wcst")
            wcb_t = cpool.tile([128, 384], BF16, tag="wcb")
            acst_t = cpool.tile([128, amax], F32, tag="acst")
            fcst_t = cpool.tile([128, 2 + nslmax], F32, tag="fcst")
            zcol_t = cpool.tile([128, ECHUNK], F32, tag="zcol")
            qe0 = [nc.sync, nc.scalar, nc.gpsimd]
            for i in range(8):
                qe0[i % 3].dma_start(
                    w3bm_t[:, 4 * i:4 * i + 4, :, :],
                    d_in["w3bm"].ap()[:, 4 * i:4 * i + 4, :, :])
            nc.sync.dma_start(w4dr_t[:], d_in["w4dr"].ap())
            nc.sync.dma_start(wcst_t[:], d_in["wcst"].ap())
            nc.sync.dma_start(acst_t[:], d_in["acst"].ap())
            nc.sync.dma_start(fcst_t[:], d_in["fcst"].ap())
            nc.gpsimd.memset(zcol_t[:], 0.0)
            nc.vector.tensor_copy(wcb_t[:], wcst_t[:])  # bf16 W5|W6|W4
            b5c = fcst_t[:, 0:1]
            b6c = fcst_t[:, 1:2]
            w5b = wcb_t[:, 0:128]
            w6b = wcb_t[:, 128:256]
            w4b = wcb_t[:, 256:384]

            def build_core(c):
                qeng = [nc.sync, nc.scalar, nc.gpsimd]
                qi = 0
                for si, it in enumerate(sched[c]):
                    e_lo = it["eoff"]
                    ln = it["L"] * it["nep"]
                    step = max(512, -(-ln // 6) // 8 * 8)
                    for p0 in range(0, ln, step):
                        pn = min(step, ln - p0)
                        qeng[qi % 3].dma_start(
                            esp_t[:, :, e_lo + p0:e_lo + p0 + pn],
                            d_in["esp"].ap()[:, :,
                                             e_lo + p0:e_lo + p0 + pn])
                        qi += 1

                chunks = []
                for si, it in enumerate(sched[c]):
                    ba = Ba[it["g"]]
                    lim = os.environ.get("KERNEL_LIMIT_BA")
                    if lim is not None:
                        ba = min(ba, int(lim))
                    for c0 in range(0, it["ne"], ECHUNK):
                        n = min(ECHUNK, it["ne"] - c0)
                        chunks.append((it, si, c0, n, ba))

                pending = []

                def emit_pending():
                    if pending:
                        pending.pop(0)()

                for it, si, c0, n, ba in chunks:
                    vb_col = fcst_t[:, 2 + si:3 + si]
                    asn = _assign(ba)
                    nact = sum(asn)
                    napair = (nact + 1) // 2
                    while len(pending) > max(0, ba - 1):
                        emit_pending()
                    pW4 = psW.tile([128, ECHUNK], F32, tag="pW4", bufs=2)
                    started = [False]

                    inflight = []   # completed ACT rp pairs awaiting W4

                    def emit_w4(last):
                        rp = inflight.pop(0)
                        nc.tensor.matmul(
                            pW4[:, :n], w4dr_t[:, 0, :, :], rp[:, :, :n],
                            start=not started[0], stop=False,
                            perf_mode=PM.DoubleRow)
                        started[0] = True
                        nc.tensor.matmul(
                            pW4[:, :n], w4dr_t[:, 1, :, :], rp[:, :, :n],
                            start=False, stop=last,
                            perf_mode=PM.DoubleRow)

                    acc = None
                    act_i = 0
                    rp_cur = None
                    ndve = ba - nact
                    w4_emitted = 0
                    for b in range(ba):
                        s_, l = b % 32, b // 32
                        ec = it["eoff"] + l * it["nep"] + c0
                        pz = psZ.tile([128, ECHUNK], F32, tag="pz", bufs=5)
                        nc.tensor.matmul(
                            pz[:, :n], w3bm_t[:, s_, :, :],
                            esp_t[:, :, ec:ec + n],
                            start=True, stop=True, perf_mode=PM.DoubleRow)
                        a_col = acst_t[:, it["aoff"] + b:
                                       it["aoff"] + b + 1]
                        if asn[b]:
                            kk = act_i % 2
                            if kk == 0:
                                rp_cur = sbp.tile([128, 2, ECHUNK], F8,
                                                  tag="rp", bufs=6,
                                                  name="rp")
                            nc.scalar.activation(
                                rp_cur[:, kk, :n], pz[:, :n],
                                AF.Relu, bias=a_col, scale=1.0)
                            act_i += 1
                            if kk == 1:
                                inflight.append(rp_cur)
                            elif act_i == nact:   # odd straggler
                                nc.gpsimd.memset(rp_cur[:, 1, :n], 0.0)
                                inflight.append(rp_cur)
                            if len(inflight) > DEPTH:
                                last = (w4_emitted == napair - 1
                                        and ndve == 0)
                                emit_w4(last)
                                w4_emitted += 1
                        else:
                            first = acc is None
                            if first:
                                acc = sbp.tile([128, ECHUNK], F32,
                                               tag="acc", bufs=2,
                                               name="acc")
                            nc.vector.scalar_tensor_tensor(
                                acc[:, :n], pz[:, :n], a_col,
                                zcol_t[:, :n] if first else acc[:, :n],
                                ALU.max, ALU.add)
                        emit_pending()
                    while inflight:
                        last = (w4_emitted == napair - 1 and ndve == 0)
                        emit_w4(last)
                        w4_emitted += 1
                    if ndve > 0:
                        # join the DVE accumulator: pW4 += bf16(acc) @ W4
                        accb = sbp.tile([128, ECHUNK], BF16, tag="accb",
                                        bufs=2, name="accb")
                        nc.scalar.activation(accb[:, :n], acc[:, :n],
                                             AF.Identity, bias=0.0,
                                             scale=1.0)
                        nc.tensor.matmul(
                            pW4[:, :n], w4b, accb[:, :n],
                            start=not started[0], stop=True)
                        started[0] = True

                    # ---- deferred out stage ----
                    def make_out(it=it, c0=c0, n=n, pW4=pW4, vb_col=vb_col):
                        st = {}

                        def s1():
                            st["r5"] = sbp.tile([128, ECHUNK], BF16,
                                                tag="r5", bufs=2, name="r5")
                            nc.vector.tensor_scalar(
                                st["r5"][:, :n], pW4[:, :n], vb_col, None,
                                ALU.add)

                        def s2():
                            st["p2"] = psZ.tile([128, ECHUNK], F32,
                                                tag="pz", bufs=5, name="p2")
                            nc.tensor.matmul(
                                st["p2"][:, :n], w5b, st["r5"][:, :n],
                                start=True, stop=True)

                        def s3():
                            st["r6"] = sbp.tile([128, ECHUNK], BF16,
                                                tag="r6", bufs=2, name="r6")
                            nc.scalar.activation(
                                st["r6"][:, :n], st["p2"][:, :n],
                                AF.Relu, bias=b5c, scale=1.0)

                        def s4():
                            st["p3"] = psZ.tile([128, ECHUNK], F32,
                                                tag="pz", bufs=5, name="p3")
                            nc.tensor.matmul(
                                st["p3"][:, :n], w6b, st["r6"][:, :n],
                                start=True, stop=True)

                        def s5():
                            o_sb = sbp.tile([128, ECHUNK], F32, tag="o",
                                            bufs=2, name="o_sb")
                            nc.scalar.activation(
                                o_sb[:, :n], st["p3"][:, :n],
                                AF.Identity, bias=b6c, scale=1.0)
                            oc = it["ooff"] + c0
                            nc.sync.dma_start(d_out.ap()[:, oc:oc + n],
                                              o_sb[:, :n])

                        return [s1, s2, s3, s4, s5]

                    pending.extend(make_out())
                while pending:
                    pending.pop(0)()

            if one_core is not None:
                build_core(int(one_core))
            else:
                for case in tc.Switch(pid, NCORES):
                    build_core(case)

    global LAST_NC, LAST_INMAPS, LAST_SCHED
    LAST_NC, LAST_INMAPS, LAST_SCHED = nc, in_maps, sched
    if os.environ.get("KERNEL_BUILD_ONLY"):
        return np.zeros((B * MAX_E, HID), np.float32)
    nc.compile()
    trace = bool(os.environ.get("KERNEL_TRACE"))
    run_maps = [in_maps[int(one_core)]] if one_core is not None else in_maps
    res = bass_utils.run_bass_kernel_spmd(
        nc, run_maps, core_ids=list(range(len(run_maps))),
        trace=trace,
        trace_cores=list(range(len(run_maps))) if trace else None,
    )
    global LAST_EXEC_NS, LAST_RESULTS
    LAST_RESULTS = res
    LAST_EXEC_NS = res.exec_time_ns

    out = np.zeros((B * MAX_E, HID), np.float32)
    core_list = [int(one_core)] if one_core is not None else range(NCORES)
    for ci, c in enumerate(core_list):
        oc = res.results[ci]["out"]
        for it in sched[c]:
            g, e0, e1 = it["g"], it["e0"], it["e1"]
            out[g * MAX_E + e0:g * MAX_E + e1] = \
                oc[:, it["ooff"]:it["ooff"] + (e1 - e0)].T
    return out


# revision 30
# speedup vs baseline: 1.1921x; 1.1416x over previous
"""Trainium2 Bass kernel for nn_CycleNet_EPD (ragged graph edge-phase decoder).

Math (per graph b, with La = edge_len[b], Ba = beta_len[b]):
  ef[e,:4]   = [x[src_e], x[dst_e]]                        (edge features)
  s[beta,:]  = sum_e |SCB[b,beta,e]| * ef[e,:]             (beta < Ba, e < La)
  emb        = relu(s@W1+b1)@W2+b2                         [Ba,64]
  A[beta,:]  = emb@W3a + b3                                [Ba,128]  (W3a=W3[:64])
  G[e,:]     = ef@W3b                                      [La,128]  (W3b=W3[64:])
  H[e,:]     = sum_{beta<Ba} relu(A[beta,:] + |SCB[b,beta,e]|*G[e,:])
  out[e,:]   = relu((H@W4 + vb)@W5+b5)@W6+b6
               vb = 64*b4 + (64-Ba)*relu(A_pad)@W4  (A_pad: padded-beta row)
  rows with e >= La are zero.

Device mapping (per graph), [h, e] layout, e-chunks of 512:
  - G is rank 4 (G = W3b^T ef), so scb_beta (x) G = W3b^T (ef (x) scb_beta).
    Host ships 32x-tiled edge features (eft32, rows 4i+k = ef[k]) and
    4x-replicated |scb| (scb32, rows 4i+k = |scb[g0+i]|), both bf16; one
    DVE multiply per 32-beta group builds EFS[4i+k,e] = ef[k,e]*|scb| rows.
    Per beta, a single K=128 bf16 matmul with masked stacked weights w3bm
    (rows 4i..4i+3 = W3b, zeros elsewhere) yields scb_beta*G in PSUM.
  - relu(+A bias) on ACT (3/4 of betas) / DVE tensor_scalar (1/4, balance).
  - The beta-sum is folded into W4: DVE pair-sums 4 consecutive r's (bf16
    2x mode) and one r_quad@W4b matmul per quad accumulates into one PSUM
    bank (start/stop across the chunk's quads).
  - out stage: W5/W6 bf16 matmuls with ACT/DVE bias epilogues; the output
    stays [h, e] (one plain DMA per chunk) and the host transposes rows
    during the unshard scatter.
  - all hot-loop matmuls bf16 (1 cyc/row); fp32 only in the tiny emb chain.

Sharding: per-core work items (graph, e0, e1); effective load model counts
columns Ba*ne plus per-slice and per-edge-column fixed overheads so cores
with many small graphs are not overloaded. One NEFF; each core's ragged
schedule sits in its own branch of a partition-id If-tree.
Host does only data movement: gather of x rows by edge_index, packing /
replication / dtype casts of inputs, and scatter of per-core outputs into
the full [B*MAX_E, HID] result (padded rows stay zero).
"""

import sys

sys.path.insert(0, "/opt/trn_rl_repo")

import ml_dtypes
import numpy as np

import concourse.bacc as bacc
import concourse.mybir as mybir
import concourse.tile as tile
from concourse import bass_utils

B, MAX_N, MAX_E, MAX_BETA = 16, 512, 1024, 64
NODE_F, HID = 2, 128
NCORES = 8
F32 = mybir.dt.float32
BF16 = mybir.dt.bfloat16
AF = mybir.ActivationFunctionType
ALU = mybir.AluOpType
NPBF16 = ml_dtypes.bfloat16

ECHUNK = 512   # e-tile for stage B / out stage (one PSUM bank)
GFIX = 3000    # planner: per-slice fixed cost (stage A), in column units
OUT_W = 5      # planner: out-stage cost per edge column, in column units


def _relu_on_dve(b):
    return b % 4 == 2


def _plan(edge_len, beta_len):
    """Per-core work items (g, e0, e1); large graphs split by edge range.

    Effective load = (Ba + OUT_W) * ne + GFIX per slice, balancing stage-B
    columns plus out-stage and per-graph fixed overheads.  Split oversized
    graphs, LPT-assign pieces, then iteratively shave edges from the max
    core onto the min core."""
    La = [max(1, min(MAX_E, int(v))) for v in edge_len]
    Ba = [max(1, min(MAX_BETA, int(v))) for v in beta_len]

    def el(g, ne):
        return (Ba[g] + OUT_W) * ne + GFIX

    total = sum(el(g, La[g]) for g in range(B))
    target = total / NCORES
    pieces = []
    for g in range(B):
        k = max(1, min(round(el(g, La[g]) / target + 0.25), -(-La[g] // 64)))
        base, rem = divmod(La[g], k)
        e0 = 0
        for j in range(k):
            ne = base + (1 if j < rem else 0)
            pieces.append((g, e0, e0 + ne))
            e0 += ne
    pieces.sort(key=lambda p: -el(p[0], p[2] - p[1]))
    cores = [[] for _ in range(NCORES)]
    loads = [0.0] * NCORES
    for p in pieces:
        c = min(range(NCORES), key=lambda i: loads[i])
        cores[c].append(p)
        loads[c] += el(p[0], p[2] - p[1])
    for _ in range(64):  # shave the max core onto the min core
        cM = max(range(NCORES), key=lambda i: loads[i])
        cm = min(range(NCORES), key=lambda i: loads[i])
        surplus = loads[cM] - loads[cm]
        best = None
        for idx, (g, e0, e1) in enumerate(cores[cM]):
            ne_mv = int((surplus / 2 - GFIX) / (Ba[g] + OUT_W))
            ne_mv = min(ne_mv, e1 - e0 - 64)
            if ne_mv >= 64 and (best is None or ne_mv > best[1]):
                best = (idx, ne_mv)
        if best is None:
            break
        idx, ne_mv = best
        g, e0, e1 = cores[cM][idx]
        cores[cM][idx] = (g, e0, e1 - ne_mv)
        cores[cm].append((g, e1 - ne_mv, e1))
        loads[cM] -= (Ba[g] + OUT_W) * ne_mv
        loads[cm] += el(g, ne_mv)
    return La, Ba, cores


def kernel(x, SCB, edge_index, edge_len, beta_len,
           W1, b1, W2, b2, W3, b3, W4, b4, W5, b5, W6, b6):
    x = np.asarray(x, np.float32)
    SCB = np.asarray(SCB, np.float32)
    edge_index = np.asarray(edge_index, np.int32)
    La, Ba, cores = _plan(np.asarray(edge_len), np.asarray(beta_len))
    ngmax = max(len(c) for c in cores)

    # ---- host-side packing (data movement only) ----
    ef_all = []
    for b in range(B):
        src = edge_index[b, 0, : La[b]]
        dst = edge_index[b, 1, : La[b]]
        ef_all.append(np.concatenate([x[b][src], x[b][dst]], axis=1))  # [La,4]

    ef_off = [[0] * ngmax for _ in range(NCORES)]
    s32_off = [[[0, 0] for _ in range(ngmax)] for _ in range(NCORES)]
    emax = 1
    s32max = 1
    for c in range(NCORES):
        eo = 0
        so = 0
        for i, (g, e0, e1) in enumerate(cores[c]):
            ef_off[c][i] = eo
            eo += La[g]
            for gi in range(-(-Ba[g] // 32)):
                s32_off[c][i][gi] = so
                so += La[g]
        emax = max(emax, eo)
        s32max = max(s32max, so)

    W3b = np.ascontiguousarray(W3[64:], np.float32)       # [4,128]
    exp64 = np.zeros((64, 128), np.float32)               # expand 32b -> 4x32p
    for i in range(64):
        exp64[i, 4 * (i % 32) : 4 * (i % 32) + 4] = 1.0
    w3bm = np.zeros((128, 32 * 128), np.float32)          # masked stacked W3b
    for i in range(32):
        w3bm[4 * i : 4 * i + 4, i * 128 : (i + 1) * 128] = W3b

    # packed constants: one fp32 tensor + one bf16 tensor -> 2 DMAs
    # fp32 [128, 518]: w1(0:64) w2(64:128) w3a(128:256) w4(256:384)
    #   ident(384:512) b1c..b6c,b4x64(512:518)
    CF = 518
    constf = np.zeros((128, CF), np.float32)
    constf[:4, 0:64] = W1
    constf[:64, 64:128] = W2
    constf[:64, 128:256] = W3[:64]
    constf[:, 256:384] = W4
    constf[:, 384:512] = np.eye(128, dtype=np.float32)
    constf[:64, 512] = np.asarray(b1, np.float32)
    constf[:64, 513] = np.asarray(b2, np.float32)
    constf[:, 514] = np.asarray(b3, np.float32)
    constf[:, 515] = 64.0 * np.asarray(b4, np.float32)
    constf[:, 516] = np.asarray(b5, np.float32)
    constf[:, 517] = np.asarray(b6, np.float32)
    # bf16 [128, 4608]: w3bm(0:4096) w4b(4096:4224) w5b(4224:4352)
    #   w6b(4352:4480) identb(4480:4608)
    CB = 4608
    constb = np.zeros((128, CB), np.float32)
    constb[:, 0:4096] = w3bm
    constb[:, 4096:4224] = W4
    constb[:, 4224:4352] = W5
    constb[:, 4352:4480] = W6
    constb[:, 4480:4608] = np.eye(128, dtype=np.float32)
    constb = constb.astype(NPBF16)

    in_maps = []
    for c in range(NCORES):
        scb_pack = np.zeros((64, ngmax * MAX_E), np.float32)
        eft32 = np.zeros((128, emax), np.float32)
        scb32 = np.zeros((128, s32max), np.float32)
        for i, (g, e0, e1) in enumerate(cores[c]):
            la = La[g]
            scb_pack[:, i * MAX_E : i * MAX_E + la] = SCB[g][:, :la]
            eft32[:, ef_off[c][i] : ef_off[c][i] + la] = \
                np.tile(ef_all[g].T, (32, 1))
            asc = np.abs(SCB[g][:, :la])
            for gi in range(-(-Ba[g] // 32)):
                gsz = min(32, Ba[g] - gi * 32)
                so = s32_off[c][i][gi]
                scb32[: 4 * gsz, so : so + la] = \
                    np.repeat(asc[gi * 32 : gi * 32 + gsz], 4, axis=0)
        in_maps.append({
            "constf": constf,
            "constb": constb,
            "scb_pack": scb_pack.astype(NPBF16),
            "eft32": eft32.astype(NPBF16),
            "scb32": scb32.astype(NPBF16),
        })

    # ---- build program ----
    nc = bacc.Bacc("TRN2", target_bir_lowering=False, debug=False,
                   num_devices=NCORES)
    d_in = {}
    for name, arr in in_maps[0].items():
        dt = BF16 if arr.dtype == NPBF16 else F32
        d_in[name] = nc.dram_tensor(name, list(arr.shape), dt,
                                    kind="ExternalInput")
    d_out = nc.dram_tensor("out", [HID, ngmax * MAX_E], F32,
                           kind="ExternalOutput")

    with tile.TileContext(nc) as tc:
        pid = nc.partition_id()
        with (
            tc.tile_pool(name="const", bufs=1) as cpool,
            tc.tile_pool(name="sbA", bufs=2) as sbA,
            tc.tile_pool(name="sbB", bufs=3) as sbB,
            tc.tile_pool(name="psG", bufs=4, space="PSUM") as psG,
            tc.tile_pool(name="psH", bufs=2, space="PSUM") as psH,
            tc.tile_pool(name="psO", bufs=1, space="PSUM") as psO,
        ):
            scb_all = cpool.tile([64, ngmax * MAX_E], BF16, tag="scb_all")
            cb = cpool.tile([128, CB], BF16, tag="constb")
            cf = cpool.tile([128, CF], F32, tag="constf")
            eftc = cpool.tile([128, emax], BF16, tag="eft32")
            s32c = cpool.tile([128, s32max], BF16, tag="scb32")
            # consts: cf + non-w3bm tail of cb first (small, needed早),
            # w3bm blocks split across queues
            nc.sync.dma_start(cf[:], d_in["constf"].ap())
            nc.scalar.dma_start(cb[:, 4096:CB], d_in["constb"].ap()[:, 4096:CB])
            for i in range(4):
                [nc.sync, nc.scalar, nc.gpsimd, nc.sync][i].dma_start(
                    cb[:, 1024 * i:1024 * (i + 1)],
                    d_in["constb"].ap()[:, 1024 * i:1024 * (i + 1)])
            cst = {
                "w1": cf[:4, 0:64], "w2": cf[:64, 64:128],
                "w3a": cf[:64, 128:256], "w4": cf[:, 256:384],
                "ident": cf[:, 384:512],
                "b1c": cf[:64, 512:513], "b2c": cf[:64, 513:514],
                "b3c": cf[:, 514:515], "b4x64": cf[:, 515:516],
                "b5c": cf[:, 516:517], "b6c": cf[:, 517:518],
                "w3bm": cb[:, 0:4096], "w4b": cb[:, 4096:4224],
                "w5b": cb[:, 4224:4352], "w6b": cb[:, 4352:4480],
                "identb": cb[:, 4480:4608],
            }

            def build_graph(c, slot, g, es, ee):
                la, ba = La[g], Ba[g]
                nech = (la + 127) // 128  # 128-e chunks for transposes / s
                goff_e = ef_off[c][slot]

                # ---- stage A ----
                scb_sb = sbA.tile([64, MAX_E], BF16, tag="scb")
                nc.scalar.activation(
                    scb_sb[:ba, :la],
                    scb_all[:ba, slot * MAX_E : slot * MAX_E + la],
                    AF.Abs, bias=0.0, scale=1.0)
                # scb_T chunks [128e, 64b] via PE transpose (for s)
                scbT_sb = sbA.tile([128, 64 * 8], BF16, tag="scbT")
                for ec in range(nech):
                    n = min(128, la - ec * 128)
                    tp = psO.tile([128, 64], BF16, tag="pM")
                    nc.tensor.transpose(
                        tp[:n, :], scb_sb[:, ec * 128 : ec * 128 + n],
                        cst["identb"][:64, :64],
                    )
                    nc.vector.tensor_copy(scbT_sb[:n, ec * 64 : ec * 64 + 64],
                                          tp[:n, :])
                # ef rows [e,4] per 128-chunk (for s)
                efr_sb = sbA.tile([128, 4 * 8], BF16, tag="efr")
                for ec in range(nech):
                    n = min(128, la - ec * 128)
                    tp2 = psO.tile([128, 4], BF16, tag="pM")
                    nc.tensor.transpose(
                        tp2[:n, :],
                        eftc[0:4, goff_e + ec * 128 : goff_e + ec * 128 + n],
                        cst["identb"][:4, :4],
                    )
                    nc.vector.tensor_copy(efr_sb[:n, ec * 4 : ec * 4 + 4],
                                          tp2[:n, :])
                # s_T[k, beta] = sum_e ef[e,k] |scb|_T[e, beta]
                ps_s = psO.tile([4, 128], F32, tag="pM")
                for ec in range(nech):
                    n = min(128, la - ec * 128)
                    nc.tensor.matmul(
                        ps_s[:, :ba],
                        efr_sb[:n, ec * 4 : ec * 4 + 4],
                        scbT_sb[:n, ec * 64 : ec * 64 + ba],
                        start=(ec == 0), stop=(ec == nech - 1),
                    )
                s_sb = sbA.tile([4, 65], F32, tag="s")
                nc.vector.memset(s_sb[:], 0.0)
                nc.vector.tensor_copy(s_sb[:, :ba], ps_s[:, :ba])
                # emb / A chain (one padded col at index ba -> A_pad)
                nb = ba + 1
                pe1 = psO.tile([64, 65], F32, tag="pM")
                nc.tensor.matmul(pe1[:, :nb], cst["w1"], s_sb[:, :nb],
                                 start=True, stop=True)
                e1_sb = sbA.tile([64, 65], F32, tag="e1")
                nc.scalar.activation(e1_sb[:, :nb], pe1[:, :nb], AF.Relu,
                                     bias=cst["b1c"], scale=1.0)
                pe2 = psO.tile([64, 65], F32, tag="pM")
                nc.tensor.matmul(pe2[:, :nb], cst["w2"], e1_sb[:, :nb],
                                 start=True, stop=True)
                e2_sb = sbA.tile([64, 65], F32, tag="e2")
                nc.scalar.activation(e2_sb[:, :nb], pe2[:, :nb], AF.Identity,
                                     bias=cst["b2c"], scale=1.0)
                pa = psO.tile([128, 65], F32, tag="pM")
                nc.tensor.matmul(pa[:, :nb], cst["w3a"], e2_sb[:, :nb],
                                 start=True, stop=True)
                A_sb = sbA.tile([128, 65], F32, tag="A")
                nc.scalar.activation(A_sb[:, :nb], pa[:, :nb], AF.Identity,
                                     bias=cst["b3c"], scale=1.0)
                # K0 = relu(A_pad); vb = (64-Ba) * K0@W4 + 64*b4
                K0_sb = sbA.tile([128, 1], F32, tag="K0")
                nc.scalar.activation(K0_sb[:], A_sb[:, ba : ba + 1], AF.Relu,
                                     bias=0.0, scale=1.0)
                pk = psO.tile([128, 1], F32, tag="pM")
                nc.tensor.matmul(pk[:], cst["w4"], K0_sb[:],
                                 start=True, stop=True)
                vb_sb = sbA.tile([128, 1], F32, tag="vb")
                nc.scalar.activation(vb_sb[:], pk[:], AF.Identity,
                                     bias=cst["b4x64"],
                                     scale=float(64 - ba))

                # ---- stage B + out stage, per 512-e chunk ----
                # quad accumulation: DVE pair-sums 4 consecutive r's (bf16,
                # 2x mode), one W4 acc-matmul per quad into pW4.
                nacc = (ba + 3) // 4
                for e0 in range(es, ee, ECHUNK):
                    n = min(ECHUNK, ee - e0)
                    ecol = goff_e + e0  # column of this chunk in eft32
                    pW4 = psH.tile([128, ECHUNK], F32, tag="H")
                    acc_i = 0
                    qr = []
                    for g0 in range(0, ba, 32):
                        gsz = min(32, ba - g0)
                        scol = s32_off[c][slot][g0 // 32] + e0
                        efs = sbB.tile([128, ECHUNK], BF16, tag="efs")
                        nc.vector.tensor_mul(efs[: 4 * gsz, :n],
                                             s32c[: 4 * gsz, scol : scol + n],
                                             eftc[: 4 * gsz, ecol : ecol + n])
                        for i in range(gsz):
                            b = g0 + i
                            pG = psG.tile([128, ECHUNK], F32, tag="pG")
                            nc.tensor.matmul(
                                pG[:, :n],
                                cst["w3bm"][:, i * 128 : (i + 1) * 128],
                                efs[:, :n], start=True, stop=True)
                            r = sbB.tile([128, ECHUNK], BF16, tag="r",
                                         bufs=6)
                            if _relu_on_dve(b):
                                nc.vector.tensor_scalar(
                                    r[:, :n], pG[:, :n],
                                    A_sb[:, b : b + 1], 0.0,
                                    ALU.add, ALU.max)
                            else:
                                nc.scalar.activation(
                                    r[:, :n], pG[:, :n], AF.Relu,
                                    bias=A_sb[:, b : b + 1], scale=1.0)
                            qr.append(r)
                            if len(qr) == 4 or b == ba - 1:
                                while len(qr) > 1:
                                    t0 = qr.pop(0)
                                    t1 = qr.pop(0)
                                    sq = sbB.tile([128, ECHUNK], BF16,
                                                  tag="rq", bufs=4)
                                    nc.vector.tensor_add(sq[:, :n],
                                                         t0[:, :n],
                                                         t1[:, :n])
                                    qr.append(sq)
                                nc.tensor.matmul(
                                    pW4[:, :n], cst["w4b"],
                                    qr.pop()[:, :n],
                                    start=(acc_i == 0),
                                    stop=(acc_i == nacc - 1))
                                acc_i += 1
                    # out stage: h = pW4 + vb; relu(h@W5+b5)@W6+b6
                    r5 = sbB.tile([128, ECHUNK], BF16, tag="r5")
                    nc.scalar.activation(r5[:, :n], pW4[:, :n], AF.Identity,
                                         bias=vb_sb[:], scale=1.0)
                    p2 = psO.tile([128, ECHUNK], F32, tag="pO")
                    nc.tensor.matmul(p2[:, :n], cst["w5b"], r5[:, :n],
                                     start=True, stop=True)
                    r6 = sbB.tile([128, ECHUNK], BF16, tag="r6")
                    nc.scalar.activation(r6[:, :n], p2[:, :n], AF.Relu,
                                         bias=cst["b5c"], scale=1.0)
                    p3 = psO.tile([128, ECHUNK], F32, tag="pO")
                    nc.tensor.matmul(p3[:, :n], cst["w6b"], r6[:, :n],
                                     start=True, stop=True)
                    o_sb = sbB.tile([128, ECHUNK], F32, tag="o")
                    nc.vector.tensor_scalar(o_sb[:, :n], p3[:, :n],
                                            cst["b6c"], None, ALU.add)
                    r0 = slot * MAX_E + e0
                    nc.sync.dma_start(d_out.ap()[:, r0 : r0 + n],
                                      o_sb[:, :n])

            def build_core(c):
                # stream inputs per slice on 3 queues so the first slice's
                # compute starts while later slices are still in flight
                qeng = [nc.sync, nc.scalar, nc.gpsimd]
                qi = 0
                for i, (g, e0, e1) in enumerate(cores[c]):
                    la = La[g]
                    r0 = i * MAX_E
                    qeng[qi % 3].dma_start(
                        scb_all[:, r0:r0 + la],
                        d_in["scb_pack"].ap()[:, r0:r0 + la])
                    qi += 1
                    eo = ef_off[c][i]
                    qeng[qi % 3].dma_start(
                        eftc[:, eo:eo + la], d_in["eft32"].ap()[:, eo:eo + la])
                    qi += 1
                    for gi in range(-(-Ba[g] // 32)):
                        so = s32_off[c][i][gi]
                        qeng[qi % 3].dma_start(
                            s32c[:, so:so + la],
                            d_in["scb32"].ap()[:, so:so + la])
                        qi += 1
                for slot, (g, e0, e1) in enumerate(cores[c]):
                    build_graph(c, slot, g, e0, e1)

            for case in tc.Switch(pid, NCORES):
                build_core(case)

    import os
    if os.environ.get("KERNEL_BUILD_ONLY"):
        return np.zeros((B * MAX_E, HID), np.float32)
    nc.compile()
    if os.environ.get("KERNEL_COMPILE_ONLY"):
        import tempfile
        neff = bass_utils.compile_bass_kernel(nc, tempfile.mkdtemp())
        print("NEFF:", neff)
        return np.zeros((B * MAX_E, HID), np.float32)
    trace = bool(os.environ.get("KERNEL_TRACE"))
    res = bass_utils.run_bass_kernel_spmd(
        nc, in_maps, core_ids=list(range(NCORES)),
        trace=trace,
        trace_cores=list(range(NCORES)) if trace else None,
    )
    global LAST_EXEC_NS, LAST_RESULTS
    LAST_RESULTS = res
    LAST_EXEC_NS = res.exec_time_ns

    out = np.zeros((B * MAX_E, HID), np.float32)
    for c in range(NCORES):
        oc = res.results[c]["out"]
        for slot, (g, e0, e1) in enumerate(cores[c]):
            out[g * MAX_E + e0 : g * MAX_E + e1] = \
                oc[:, slot * MAX_E + e0 : slot * MAX_E + e1].T
    return out

